# revision 41
# baseline (speedup 1.0000x reference)
"""BMOJO attention (sliding-window + fading memory, joint softmax) on 8 TRN2
NeuronCores via Bass/Tile.

Sharding: tensor-parallel over heads — core c owns q-heads {2c, 2c+1} and kv
head c for both batches and both projection paths; each core computes a partial
output through its Wo column shard and the host sums the 8 partials.

Math (per core, all matmuls bf16 with fp32 PSUM accumulation):
  1. qkv = x @ Wcat.T + bcat  for both paths (Wcat = [Wq_sh; Wk_sh; Wv_sh])
  2. rmsnorm scales r = rsqrt(mean(x^2)+eps), computed as exp(-0.5*ln(ms))
     so every ACT function the kernel uses ({Square, Ln, Exp, Copy}) lives
     in one activation-table set (gamma folded into rope tables); rope
     applied in the natural [t, d] layout with host-folded cos/sin tables
     (cg = cos*g, sg = sign*shift(g)*sin); rq applied to q, rk folded into
     the exp() scale of the score pass.
  3. scores computed transposed sT[j, i] = k~ @ q~.T so the softmax exp tiles
     feed the PV matmul as the stationary operand without any p-transpose.
     Max-free softmax: p = exp(SCALE*rk*s), 0/1 block masks after exp.
  4. PV in [i, e] with a ones-column appended to V: the PSUM accumulator picks
     up the joint (in-window + fading) softmax denominator for free.
  5. attn normalized, PE-transposed, then attnT @ WoT_shard -> partial out.

I/O: inputs are packed into two DRAM tensors (xtab = xT chunks + rope tables,
wpack = both Wcat shards + WoT shard + biases) to minimize per-launch
dispatch cost and DMA instruction count.
"""
import numpy as np
import ml_dtypes

import concourse.bass as bass
import concourse.tile as tile
from concourse import bacc, mybir
from concourse import bass_utils
from concourse.masks import make_identity

BFNP = ml_dtypes.bfloat16
F32 = mybir.dt.float32
BF16 = mybir.dt.bfloat16

B, S, DM = 2, 1024, 2048
H, HKV, D = 16, 8, 128
W = 256
EPS = 1e-6
SCALE = D ** -0.5
P = 128
T = B * S           # 2048 flattened tokens
NT = T // P         # 16 t-chunks
ND = DM // P        # 16 d-chunks
NB = S // P         # 8 s-blocks per batch
N_CORES = 8

AluOp = mybir.AluOpType
ACT_FN = mybir.ActivationFunctionType


def _ic_width(bj):
    # in-window scores for key block bj cover query blocks {bj, bj+1, bj+2}
    return min(P * (bj + 3), S) - P * bj


def _f_width(bj):
    # fading scores for key block bj cover query blocks {bj+2 .. NB-1}
    return max(0, S - P * (bj + 2))


def _build_tile_kernel(tc):
    nc = tc.nc

    # merged streams: xtab[tt] = [xT chunk (ND*P) | rope tables (2*2*384)]
    # (fewer DRAM tensors -> lower per-call dispatch cost; fewer, larger DMAs)
    xtab = nc.dram_tensor("xtab", (NT, P, ND * P + 1536), BF16,
                          kind="ExternalInput").ap()
    # wpack: [wcat0 (ND*512) | wcat1 (ND*512) | woT (2*DM) | bcat0|bcat1 @p0]
    wpack = nc.dram_tensor("wpack", (P, 21504), BF16, kind="ExternalInput").ap()
    out = nc.dram_tensor("out", (T, DM), BF16, kind="ExternalOutput").ap()

    def wslice(c0, c1):
        return wpack[:, c0:c1].rearrange("p (a b) -> p a b", b=512)

    consts = tc.alloc_tile_pool(name="consts", bufs=1)
    weights = tc.alloc_tile_pool(name="weights", bufs=1)
    resident = tc.alloc_tile_pool(name="resident", bufs=1)
    xstream = tc.alloc_tile_pool(name="xstream", bufs=4)
    work = tc.alloc_tile_pool(name="work", bufs=4)
    stats = tc.alloc_tile_pool(name="stats", bufs=4)
    expool = tc.alloc_tile_pool(name="expool", bufs=2)
    outsb = tc.alloc_tile_pool(name="outsb", bufs=6)
    psum_proj = tc.alloc_tile_pool(name="psum_proj", bufs=2, space="PSUM")
    psum_sc = tc.alloc_tile_pool(name="psum_sc", bufs=3, space="PSUM")
    psum_pv = tc.alloc_tile_pool(name="psum_pv", bufs=2, space="PSUM")
    psum_tr = tc.alloc_tile_pool(name="psum_tr", bufs=1, space="PSUM")

    # constants
    ident = consts.tile([P, P], BF16)
    make_identity(nc, ident)
    t1m = consts.tile([P, P], BF16)   # keep i' >= j'  (partition = j', free = i')
    nc.gpsimd.memset(t1m, 1.0)
    nc.gpsimd.affine_select(out=t1m, in_=t1m, compare_op=AluOp.is_ge, fill=0.0,
                            base=0, pattern=[[1, P]], channel_multiplier=-1)
    t2m = consts.tile([P, P], BF16)   # keep i' < j'  i.e. (j' - i' - 1) >= 0
    nc.gpsimd.memset(t2m, 1.0)
    nc.gpsimd.affine_select(out=t2m, in_=t2m, compare_op=AluOp.is_ge, fill=0.0,
                            base=-1, pattern=[[-1, P]], channel_multiplier=1)
    ones1 = consts.tile([1, P], BF16)
    nc.vector.memset(ones1, 1.0)
    eps_q = consts.tile([P, 1], F32)
    nc.vector.memset(eps_q, EPS)
    eps_k = consts.tile([P, 1], F32)
    nc.vector.memset(eps_k, D * EPS)

    # big resident inputs
    wsb = [weights.tile([P, ND, 512], BF16, name=f"wsb{p}") for p in range(2)]
    bsb = [weights.tile([1, 512], BF16, name=f"bsb{p}") for p in range(2)]
    nc.sync.dma_start(out=bsb[0], in_=wpack[0:1, 20480:20992])
    nc.sync.dma_start(out=wsb[0][:, 0:1], in_=wslice(0, 512))
    nc.sync.dma_start(out=wsb[0][:, 1:4], in_=wslice(512, 2048))
    wosb = weights.tile([P, 2, DM], BF16)

    # per-path residents: qkT[path]: [d=128, slot(q0,q1,k), t], v(+ones)
    qkT = [resident.tile([P, 3, T], BF16, name=f"qkT{p}") for p in range(2)]
    vsb = [resident.tile([P, NT, P + 1], BF16, name=f"vsb{p}") for p in range(2)]
    # rall[:, tt, p*3:p*3+3] = (rq0, rq1, rk_scaled) for chunk tt, path p
    rall = resident.tile([P, NT, 6], F32, name="rall")
    for p in range(2):
        nc.vector.memset(vsb[p][:, :, P:P + 1], 1.0)
    attnT = [resident.tile([P, T], BF16, name=f"attnT{h}") for h in range(2)]

    # ---------------- stage 1: projections + rmsnorm + rope + transpose ----
    def stage1(tt, pre=None, paths=(0, 1)):
        if pre is None:
            xt_tab = xstream.tile([P, ND * P + 1536], BF16, tag="xt")
            # x part first: the rope tables are consumed one psum-latency
            # later, so splitting keeps the projection fed when the stream
            # is bandwidth-bound (kernel start)
            nc.sync.dma_start(out=xt_tab[:, 0:ND * P],
                              in_=xtab[tt][:, 0:ND * P])
            nc.sync.dma_start(out=xt_tab[:, ND * P:],
                              in_=xtab[tt][:, ND * P:])
        else:
            xt_tab = pre
        xt = xt_tab[:, 0:ND * P].rearrange("p (a b) -> p a b", b=P)
        tab = xt_tab[:, ND * P:].rearrange("p (a b c) -> p a b c", a=2, b=2)

        # dead fading-path columns: its q is never used by queries i < W
        # (s-blocks 0,1) and its k/v never serve keys j > S-W (s-blocks 6,7)
        sblk = tt % NB
        p1c = (256, 512) if sblk <= 1 else (0, 256) if sblk >= NB - 2 \
            else (0, 512)

        pss = {}
        ssq6 = stats.tile([P, 6], F32, tag="ssq6")
        for p in paths:
            c0, c1 = (0, 512) if p == 0 else p1c
            ps = psum_proj.tile([P, 512], F32, tag="proj")
            pss[p] = ps
            # bias via K=1 matmul, then accumulate the 16 d-chunks
            nc.tensor.matmul(ps[:, c0:c1], lhsT=ones1, rhs=bsb[p][:, c0:c1],
                             start=True, stop=False)
            for dd in range(ND):
                nc.tensor.matmul(ps[:, c0:c1], lhsT=xt[:, dd, :],
                                 rhs=wsb[p][:, dd, c0:c1],
                                 start=False, stop=(dd == ND - 1))
        for p in paths:
            c0, sc1 = (0, 384) if p == 0 else (p1c[0], min(p1c[1], 384))
            s0, s1 = c0 // P, sc1 // P
            ps = pss[p]
            # squares (one ACT op; DVE can't — walrus allows only one PSUM
            # input per instruction), then per-head-slot sums (one DVE reduce)
            sqsb = work.tile([P, 3, P], BF16, tag="sqsb")
            nc.scalar.activation(
                out=sqsb[:, s0:s1, :].rearrange("p a b -> p (a b)"),
                in_=ps[:, c0:sc1], func=ACT_FN.Square)
            nc.vector.tensor_reduce(out=ssq6[:, p * 3 + s0:p * 3 + s1],
                                    in_=sqsb[:, s0:s1, :],
                                    axis=mybir.AxisListType.X, op=AluOp.add)
        # rq = 1/sqrt(ssq/D + eps); rk_scaled = SCALE*rk = 1/sqrt(ssq + D*eps)
        # via exp(-0.5*ln(.)) so every ACT func ({Square, Ln, Exp, Copy}) is
        # in one table set (natural_log_exp_and_others) -- no table thrash
        lg6 = stats.tile([P, 6], F32, tag="sq6")
        if len(paths) == 1:
            p = paths[0]
            c0, sc1 = (0, 384) if p == 0 else (p1c[0], min(p1c[1], 384))
            s0, s1 = c0 // P, sc1 // P
            b = p * 3
            qa, qb = s0, min(s1, 2)
            if qb > qa:
                nc.scalar.activation(out=lg6[:, b + qa:b + qb],
                                     in_=ssq6[:, b + qa:b + qb],
                                     func=ACT_FN.Ln, scale=1.0 / D, bias=eps_q)
            if s1 == 3:
                nc.scalar.activation(out=lg6[:, b + 2:b + 3],
                                     in_=ssq6[:, b + 2:b + 3],
                                     func=ACT_FN.Ln, scale=1.0, bias=eps_k)
            nc.scalar.activation(out=rall[:, tt, b + s0:b + s1],
                                 in_=lg6[:, b + s0:b + s1],
                                 func=ACT_FN.Exp, scale=-0.5)
        elif p1c == (0, 512):
            ssq_v = ssq6.rearrange("p (a b) -> p a b", a=2, b=3)
            lg_v = lg6.rearrange("p (a b) -> p a b", a=2, b=3)
            nc.scalar.activation(out=lg_v[:, :, 0:2], in_=ssq_v[:, :, 0:2],
                                 func=ACT_FN.Ln, scale=1.0 / D, bias=eps_q)
            nc.scalar.activation(out=lg_v[:, :, 2:3], in_=ssq_v[:, :, 2:3],
                                 func=ACT_FN.Ln, scale=1.0, bias=eps_k)
            nc.scalar.activation(out=rall[:, tt, :], in_=lg6,
                                 func=ACT_FN.Exp, scale=-0.5)
        else:
            nc.scalar.activation(out=lg6[:, 0:2], in_=ssq6[:, 0:2],
                                 func=ACT_FN.Ln, scale=1.0 / D, bias=eps_q)
            nc.scalar.activation(out=lg6[:, 2:3], in_=ssq6[:, 2:3],
                                 func=ACT_FN.Ln, scale=1.0, bias=eps_k)
            if p1c[0] == 0:   # fading path q-only
                nc.scalar.activation(out=lg6[:, 3:5], in_=ssq6[:, 3:5],
                                     func=ACT_FN.Ln, scale=1.0 / D,
                                     bias=eps_q)
                nc.scalar.activation(out=rall[:, tt, 0:5], in_=lg6[:, 0:5],
                                     func=ACT_FN.Exp, scale=-0.5)
            else:             # fading path kv-only
                nc.scalar.activation(out=lg6[:, 5:6], in_=ssq6[:, 5:6],
                                     func=ACT_FN.Ln, scale=1.0, bias=eps_k)
                nc.scalar.activation(out=rall[:, tt, 0:3], in_=lg6[:, 0:3],
                                     func=ACT_FN.Exp, scale=-0.5)
                nc.scalar.activation(out=rall[:, tt, 5:6], in_=lg6[:, 5:6],
                                     func=ACT_FN.Exp, scale=-0.5)

        for p in paths:
            c0, sc1 = (0, 384) if p == 0 else (p1c[0], min(p1c[1], 384))
            w = sc1 - c0
            s0, s1 = c0 // P, sc1 // P
            ps = pss[p]
            # rope over the live head-slots at once
            cg = tab[:, p, 0, c0:sc1]
            sg = tab[:, p, 1, c0:sc1]
            ra = work.tile([P, 384], BF16, tag="ra")
            nc.vector.tensor_tensor(out=ra[:, c0:sc1], in0=ps[:, c0:sc1],
                                    in1=cg, op=AluOp.mult)
            # rotate-half read of the psum q/k: one op via a reversed-half AP
            psw = ps[:, c0:sc1]
            pr_sw = bass.AP(tensor=psw.tensor, offset=psw.offset + 64,
                            ap=[list(psw.ap[0]), [128, w // P], [-64, 2],
                                [1, 64]])
            rb = work.tile([P, 384], BF16, tag="rb")
            nc.vector.tensor_tensor(
                out=rb[:, c0:sc1].rearrange("p (h s d) -> p h s d",
                                            h=w // P, s=2, d=64),
                in0=pr_sw,
                in1=sg.rearrange("p (h s d) -> p h s d", h=w // P, s=2, d=64),
                op=AluOp.mult)
            qkn = work.tile([P, 384], BF16, tag="qkn")
            nc.vector.tensor_add(out=qkn[:, c0:sc1], in0=ra[:, c0:sc1],
                                 in1=rb[:, c0:sc1])
            if s0 == 0:
                til = work.tile([P, 256], BF16, tag="til")
                for h in range(2):
                    nc.vector.tensor_scalar_mul(
                        out=til[:, h * P:(h + 1) * P],
                        in0=qkn[:, h * P:(h + 1) * P],
                        scalar1=rall[:, tt, p * 3 + h:p * 3 + h + 1])
            # v (+ ones col already set)
            if p == 0 or p1c[1] == 512:
                nc.scalar.copy(out=vsb[p][:, tt, 0:P], in_=ps[:, 384:512])

            # transpose the live slots into one psum bank, then copy out
            tr = psum_tr.tile([P, 512], BF16, tag="tr")
            if s0 == 0:
                nc.tensor.transpose(tr[:, 0:P], til[:, 0:P], ident)
                nc.tensor.transpose(tr[:, P:2 * P], til[:, P:2 * P], ident)
            if s1 == 3:
                nc.tensor.transpose(tr[:, 2 * P:3 * P], qkn[:, 2 * P:3 * P],
                                    ident)
            nc.vector.tensor_copy(
                out=qkT[p][:, s0:s1, tt * P:(tt + 1) * P],
                in_=tr[:, c0:sc1].rearrange("p (h t) -> p h t", h=w // P))

    # ---------------- stage 2: attention, both heads of one batch ----------
    def bc(mask, n):
        # broadcast a [128,128] mask across n block-rows via a 0-step AP
        return bass.AP(tensor=mask.tensor, offset=mask.offset,
                       ap=[mask.ap[0], [0, n]] + list(mask.ap[1:]))

    def stage2_block(b, bj, exp_ic, exp_f, group_tr=None):
        """scores + exp + masks + PV for key/query block bj of batch b."""
        kt_ic = qkT[0][:, 2, (b * S + bj * P):(b * S + (bj + 1) * P)]
        kt_f = qkT[1][:, 2, (b * S + bj * P):(b * S + (bj + 1) * P)]
        w_ic = _ic_width(bj)
        i0 = b * S + bj * P
        wf = _f_width(bj)
        i0f = b * S + P * (bj + 2)
        for h in range(2):
            pssc = psum_sc.tile([P, 512], F32, tag="sc")
            nc.tensor.matmul(pssc[:, 0:w_ic], lhsT=kt_ic,
                             rhs=qkT[0][:, h, i0:i0 + w_ic],
                             start=True, stop=True)
            nc.scalar.activation(out=exp_ic[:, h, bj, 0:w_ic],
                                 in_=pssc[:, 0:w_ic], func=ACT_FN.Exp,
                                 scale=rall[:, b * NB + bj, 2:3])
            for c0 in range(0, wf, 512):
                wc = min(512, wf - c0)
                psf = psum_sc.tile([P, 512], F32, tag="sc")
                nc.tensor.matmul(psf[:, 0:wc], lhsT=kt_f,
                                 rhs=qkT[1][:, h, i0f + c0:i0f + c0 + wc],
                                 start=True, stop=True)
                nc.scalar.activation(
                    out=exp_f[:, h, bj, c0:c0 + wc], in_=psf[:, 0:wc],
                    func=ACT_FN.Exp, scale=rall[:, b * NB + bj, 5:6])
            # diag mask on DVE (it gates PV's final matmul — shortest chain);
            # the other masks go to the idle GPSIMD engine, per head right
            # after its exps so PV(h0) need not wait for h1's scores
            dia = exp_ic[:, h, bj, 0:P]
            nc.vector.tensor_tensor(out=dia, in0=dia, in1=t1m, op=AluOp.mult)
            if w_ic > 256:
                ic2 = exp_ic[:, h, bj, 256:384]
                nc.gpsimd.tensor_tensor(out=ic2, in0=ic2, in1=t2m,
                                        op=AluOp.mult)
            if wf > 0:
                f2 = exp_f[:, h, bj, 0:P]
                nc.gpsimd.tensor_tensor(out=f2, in0=f2, in1=t1m,
                                        op=AluOp.mult)

        # PV for query block bi == bj; diagonal (freshest exp) last
        bi = bj
        for h in range(2):
            pv = psum_pv.tile([P, P + 1], F32, tag="pv")
            mms = []
            for bjj in range(0, bi - 1):
                mms.append((exp_f[:, h, bjj, (bi - bjj - 2) * P:(bi - bjj - 1) * P],
                            vsb[1][:, b * NB + bjj, :]))
            for bjj in range(max(0, bi - 2), bi):
                mms.append((exp_ic[:, h, bjj, (bi - bjj) * P:(bi - bjj + 1) * P],
                            vsb[0][:, b * NB + bjj, :]))
            mms.append((exp_ic[:, h, bi, 0:P], vsb[0][:, b * NB + bi, :]))
            for mi, (lhsT, rhs) in enumerate(mms):
                nc.tensor.matmul(pv, lhsT=lhsT, rhs=rhs,
                                 start=(mi == 0), stop=(mi == len(mms) - 1))
            rl = stats.tile([P, 1], F32, tag="rl")
            nc.vector.reciprocal(rl, pv[:, P:P + 1])
            anorm = work.tile([P, P], BF16, tag="anorm")
            nc.vector.tensor_scalar_mul(out=anorm, in0=pv[:, 0:P], scalar1=rl)
            if group_tr is None:
                atr = psum_tr.tile([P, P], BF16, tag="tr")
                nc.tensor.transpose(atr, anorm, ident)
                nc.vector.tensor_copy(
                    out=attnT[h][:, b * S + bi * P:b * S + (bi + 1) * P],
                    in_=atr)
            else:
                # group 4 transposed blocks per psum bank; one copy per group
                nc.tensor.transpose(
                    group_tr[h][:, (bi % 4) * P:(bi % 4 + 1) * P], anorm, ident)
                if bi % 4 == 3:
                    t0 = b * S + (bi - 3) * P
                    nc.vector.tensor_copy(out=attnT[h][:, t0:t0 + 512],
                                          in_=group_tr[h])

    # ---------------- stage 3: output projection ---------------------------
    def stage3(tt_range, copy_engine="dve", pool=None):
        for tt in tt_range:
            for oo in range(4):
                if pool is None:
                    po = psum_proj.tile([P, 512], F32, tag="proj")
                elif pool == "alt":
                    if oo % 2 == 0:
                        po = psum_proj.tile([P, 512], F32, tag="proj")
                    else:
                        po = psum_sc.tile([P, 512], F32, tag="sc",
                                          name=f"po{tt}_{oo}")
                else:
                    po = pool.tile([P, 512], F32, tag="sc", name=f"po{tt}_{oo}")
                for h in range(2):
                    nc.tensor.matmul(po, lhsT=attnT[h][:, tt * P:(tt + 1) * P],
                                     rhs=wosb[:, h, oo * 512:(oo + 1) * 512],
                                     start=(h == 0), stop=(h == 1))
                ot = outsb.tile([P, 512], BF16, tag="ot")
                if copy_engine == "fine" and oo == 3:
                    # tail chunk: halve the copy latency (both engines in
                    # parallel) and the final-DMA size
                    nc.vector.tensor_copy(out=ot[:, 0:256], in_=po[:, 0:256])
                    nc.scalar.copy(out=ot[:, 256:512], in_=po[:, 256:512])
                    c0 = oo * 512
                    nc.sync.dma_start(
                        out=out[tt * P:(tt + 1) * P, c0:c0 + 256],
                        in_=ot[:, 0:256])
                    nc.sync.dma_start(
                        out=out[tt * P:(tt + 1) * P, c0 + 256:c0 + 512],
                        in_=ot[:, 256:512])
                    continue
                if copy_engine == "dve":
                    nc.vector.tensor_copy(out=ot, in_=po)
                elif copy_engine == "act":
                    nc.scalar.copy(out=ot, in_=po)
                else:  # both
                    if oo % 2 == 0:
                        nc.vector.tensor_copy(out=ot, in_=po)
                    else:
                        nc.scalar.copy(out=ot, in_=po)
                nc.sync.dma_start(
                    out=out[tt * P:(tt + 1) * P, oo * 512:(oo + 1) * 512], in_=ot)

    # ---- emission order tuned for overlap ---------------------------------
    # Cold start is bandwidth-bound on (wcat0 | x | wcat1): prefetch x for
    # the first 3 chunks and run their in-context-path projections first so
    # the PE has dense work while the fading path's weights stream in.
    xts = []
    for t3 in range(3):
        xt = xstream.tile([P, ND * P + 1536], BF16, tag="xt")
        nc.sync.dma_start(out=xt[:, 0:ND * P], in_=xtab[t3][:, 0:ND * P])
        if t3 == 0:
            nc.sync.dma_start(out=wsb[0][:, 4:], in_=wslice(2048, 8192))
        nc.sync.dma_start(out=xt[:, ND * P:], in_=xtab[t3][:, ND * P:])
        xts.append(xt)
    nc.sync.dma_start(out=wsb[1][:, 0:4], in_=wslice(8192, 10240))
    nc.sync.dma_start(out=bsb[1], in_=wpack[0:1, 20992:21504])
    nc.sync.dma_start(out=wsb[1][:, 4:], in_=wslice(10240, 16384))

    for t3 in range(3):
        stage1(t3, pre=xts[t3], paths=(0,))
    for t3 in range(3):
        stage1(t3, pre=xts[t3], paths=(1,))
    for tt in range(3, NB):
        stage1(tt)
    # off the startup critical path
    nc.sync.dma_start(out=wosb,
                      in_=wpack[:, 16384:20480].rearrange("p (a b) -> p a b",
                                                          a=2))

    # batch-0 attention, software-pipelined with batch-1 projections and the
    # first Wo chunks as dense PE filler between exp-gated score/PV bursts
    exp0_ic = expool.tile([P, 2, NB, 384], BF16, tag="exp_ic")
    exp0_f = expool.tile([P, 2, 6, 768], BF16, tag="exp_f")
    s3_after0 = {3: [0, 1], 4: [2, 3], 5: [4], 6: [5]}
    s1_after0 = {0: [8], 1: [9], 2: [10], 3: [11], 4: [12, 13], 5: [14],
                 6: [15]}
    for bj in range(NB):
        stage2_block(0, bj, exp0_ic, exp0_f)
        for tt in s1_after0.get(bj, []):
            stage1(tt)
        for tt in s3_after0.get(bj, []):
            stage3([tt], copy_engine="both", pool=psum_sc)

    # batch-1 attention, with its Wo chunks as filler (bi done at bj >= bi)
    exp1_ic = expool.tile([P, 2, NB, 384], BF16, tag="exp_ic")
    exp1_f = expool.tile([P, 2, 6, 768], BF16, tag="exp_f")
    s3_after1 = {0: [6, 7], 1: [8], 2: [9], 3: [10], 4: [11], 5: [12, 13],
                 6: [14], 7: [15]}
    for bj in range(NB):
        stage2_block(1, bj, exp1_ic, exp1_f)
        for tt in s3_after1.get(bj, []):
            stage3([tt], copy_engine="both",
                   pool=(psum_sc if bj == 7 else None))

    for pool in reversed((consts, weights, resident, xstream, work,
                          stats, expool, outsb, psum_proj, psum_sc, psum_pv,
                          psum_tr)):
        pool.release()


_NC_CACHE = {}


def _get_nc():
    if "nc" not in _NC_CACHE:
        nc = bacc.Bacc("TRN2", target_bir_lowering=False, debug=False,
                       num_devices=N_CORES)
        with tile.TileContext(nc) as tc:
            _build_tile_kernel(tc)
        # Pre-place one ACT table load of the combined {ln, exp, square, copy}
        # set at the top of the program. Every activation func this kernel
        # uses is in natural_log_exp_and_others, but bacc's automatic
        # insertion picks the first set containing each func (natural_log for
        # Ln, exp_and_others for Exp), which thrashes the table RAMs at every
        # Ln/Exp boundary (~1.3 us per reload). With this load dominating all
        # activations, the fixpoint pass inserts nothing.
        from concourse.hw_specs import get_activation_tables
        tables = list(get_activation_tables(nc.m.arch))
        inst = mybir.InstLoadActFuncSet(
            name=nc.get_next_instruction_name(), ins=[], outs=[])
        inst.engine = mybir.EngineType.Activation
        inst.act_func_set_id = tables.index("natural_log_exp_and_others")
        nc.register_instruction(inst)
        nc.main_func.blocks[0].instructions.insert(0, inst)
        nc.compile()
        _NC_CACHE["nc"] = nc
    return _NC_CACHE["nc"]


def _prep_in_maps(inputs):
    f32 = np.float32
    x = np.asarray(inputs["hidden_states"], f32).reshape(T, DM)
    cos = np.asarray(inputs["cos"], f32).reshape(T, D)
    sin = np.asarray(inputs["sin"], f32).reshape(T, D)

    xT = np.ascontiguousarray(x.T)
    xTt = np.ascontiguousarray(
        xT.reshape(ND, P, NT, P).transpose(2, 1, 0, 3))

    sign = np.concatenate([-np.ones(64, f32), np.ones(64, f32)])

    def fold(g):
        g = np.asarray(g, f32)
        cg = cos * g[None, :]
        sg = sin * (sign * np.concatenate([g[64:], g[:64]]))[None, :]
        return cg, sg

    # tabs identical for every core (gammas are global)
    tabs = np.empty((T, 2, 2, 384), f32)
    for p, (gq_name, gk_name) in enumerate([("gq", "gk"), ("gq2", "gk2")]):
        cgq, sgq = fold(inputs[gq_name])
        cgk, sgk = fold(inputs[gk_name])
        tabs[:, p, 0, :] = np.concatenate([cgq, cgq, cgk], 1)
        tabs[:, p, 1, :] = np.concatenate([sgq, sgq, sgk], 1)
    xtab = np.concatenate([xTt.reshape(NT, P, ND * P),
                           tabs.reshape(NT, P, 1536)], axis=2).astype(BFNP)

    Wo = np.asarray(inputs["Wo"], f32)

    in_maps = []
    for c in range(N_CORES):
        wp = np.zeros((P, 21504), f32)
        for p, names in enumerate([("Wq", "bq", "Wk", "bk", "Wv", "bv"),
                                   ("Wq2", "bq2", "Wk2", "bk2", "Wv2", "bv2")]):
            Wq, bq, Wk, bk, Wv, bv = (np.asarray(inputs[n], f32) for n in names)
            Wcat = np.concatenate([Wq[c * 256:(c + 1) * 256],
                                   Wk[c * P:(c + 1) * P],
                                   Wv[c * P:(c + 1) * P]], 0)      # [512, DM]
            wcatT = np.ascontiguousarray(Wcat.T)                    # [DM, 512]
            wp[:, p * 8192:(p + 1) * 8192] = np.ascontiguousarray(
                wcatT.reshape(ND, P, 512).transpose(1, 0, 2)).reshape(P, 8192)
            bcat = np.concatenate([bq[c * 256:(c + 1) * 256],
                                   bk[c * P:(c + 1) * P],
                                   bv[c * P:(c + 1) * P]])
            wp[0, 20480 + p * 512:20480 + (p + 1) * 512] = bcat
        woT = np.ascontiguousarray(Wo[:, c * 256:(c + 1) * 256].T)  # [256, DM]
        wp[:, 16384:20480] = np.ascontiguousarray(
            woT.reshape(2, P, DM).transpose(1, 0, 2)).reshape(P, 4096)
        in_maps.append({"xtab": xtab, "wpack": wp.astype(BFNP)})
    return in_maps


def kernel(**inputs) -> np.ndarray:
    nc = _get_nc()
    in_maps = _prep_in_maps(inputs)
    res = bass_utils.run_bass_kernel_spmd(nc, in_maps, core_ids=list(range(N_CORES)))
    total = np.zeros((T, DM), np.float32)
    for c in range(N_CORES):
        total += res.results[c]["out"].astype(np.float32)
    return total.reshape(B, S, DM)

